# revision 2
# baseline (speedup 1.0000x reference)
#!/usr/bin/env python
"""Tensor-parallel fused attention kernel v8 for Trainium2 (8 NeuronCores).

Sharding: one KV head (+ its 4 grouped Q heads) per core.
 - int8 quantization of x sharded by token: core c quantizes tokens
   [256c, 256c+256) exactly (f32 absmax); int8 xq shared via AllGather.
 - w_qkv column-parallel (bf16), attention local per core, w_o row-parallel
   (bf16); per-core partial y (bf16) summed on host.
 - RMSNorm stats (ssq, max(z^2)) per 512-chunk AllGathered (tiny), applied
   one chunk later so collective latency hides under compute.
 - Main loop uses 512-token chunks => N=512 matmuls (amortizes LDWEIGHTS,
   which this toolchain emits per-matmul without overlap).
"""
import sys
sys.path.insert(0, '/opt/trn_rl_repo')

import numpy as np
from contextlib import ExitStack

import concourse.bass as bass
import concourse.bacc as bacc_mod
import concourse.tile as tile
import concourse.mybir as mybir
from concourse import bass_isa

F32 = mybir.dt.float32
F32R = mybir.dt.float32r
BF16 = mybir.dt.bfloat16
I8 = mybir.dt.int8
AF = mybir.ActivationFunctionType
OP = mybir.AluOpType
AX = mybir.AxisListType

DIM = 4096
NH = 32
NKV = 8
HPG = 4          # q heads per kv head (per core)
HD = 128
NCORES = 8
JQ = HPG * HD    # 512 local q rows
JL = JQ + 2 * HD # 768 local qkv rows
CQ = 256         # tokens quantized per core
CTA = 512        # tokens per main-loop chunk
THETA = 500000.0
EPS = 1e-5
SCALE = float(HD) ** -0.5
MAGIC = float(3 << 22)
DCH = DIM // 128  # 32 d-chunks
SZB = 128 * DCH * CQ  # xq bytes per rank in the merged AG payload


def build_kernel(T=2048, phase=99):
    NCH = T // CTA           # 4 chunks
    KB = CTA // 128          # 4 key blocks per chunk

    nc = bacc_mod.Bacc("TRN2", num_devices=NCORES)

    # ---- I/O -------------------------------------------------------------
    xtm_d = nc.dram_tensor("xtm", [2, 128, DIM], F32, kind="ExternalInput")
    xt_d = nc.dram_tensor("xt", [DIM, CQ], F32, kind="ExternalInput")
    wq_d = nc.dram_tensor("wq", [DIM, JL], BF16, kind="ExternalInput")
    wo_d = nc.dram_tensor("wo", [JQ, DIM], BF16, kind="ExternalInput")
    selrms_d = nc.dram_tensor("selrms", [2, HPG, 128], F32R, kind="ExternalInput")
    onesh_d = nc.dram_tensor("onesh", [128, 2, 2], BF16, kind="ExternalInput")
    ones8_d = nc.dram_tensor("ones8", [NCORES, 1], F32R, kind="ExternalInput")
    onesf_d = nc.dram_tensor("onesf", [128, 1], F32R, kind="ExternalInput")
    onesr_d = nc.dram_tensor("onesr", [1, 128], F32R, kind="ExternalInput")
    cosf_d = nc.dram_tensor("cosf", [128, T], F32, kind="ExternalInput")
    sinf_d = nc.dram_tensor("sinf", [128, T], F32, kind="ExternalInput")
    mask_d = nc.dram_tensor("maskt", [128, CTA // 128, CTA], BF16,
                            kind="ExternalInput")
    id_d = nc.dram_tensor("ident", [128, 128], F32, kind="ExternalInput")
    swap_d = nc.dram_tensor("swap64", [128, 128], F32R, kind="ExternalInput")
    yt_d = nc.dram_tensor("yt", [DIM, T], BF16, kind="ExternalOutput")

    with ExitStack() as ctx:
        tc = ctx.enter_context(tile.TileContext(nc))
        persist = ctx.enter_context(tc.tile_pool(name="persist", bufs=1))
        work = ctx.enter_context(tc.tile_pool(name="work", bufs=2))
        pwork = ctx.enter_context(tc.tile_pool(name="pwork", bufs=3))
        dram = ctx.enter_context(tc.tile_pool(name="dram", bufs=1, space="DRAM"))
        ps_a = ctx.enter_context(tc.tile_pool(name="ps_a", bufs=2, space="PSUM"))
        ps_b = ctx.enter_context(tc.tile_pool(name="ps_b", bufs=2, space="PSUM"))
        l_ps = ctx.enter_context(tc.tile_pool(name="l_ps", bufs=1, space="PSUM"))
        misc_ps = ctx.enter_context(tc.tile_pool(name="misc_ps", bufs=2, space="PSUM"))

        # ---- persistent tiles -------------------------------------------
        wq_sb = persist.tile([128, DCH, JL], BF16)
        K_sb = persist.tile([128, T], F32R)
        V_sb = persist.tile([128, T // 128, HD], BF16)
        selrms_sb = persist.tile([2, HPG, 128], F32R)
        nc.scalar.dma_start(selrms_sb[:], selrms_d.ap())
        onesh_sb = persist.tile([128, 2, 2], BF16)
        nc.scalar.dma_start(onesh_sb[:], onesh_d.ap())
        ones8_sb = persist.tile([NCORES, 1], F32R)
        nc.scalar.dma_start(ones8_sb[:], ones8_d.ap())
        onesf_sb = persist.tile([128, 1], F32R)
        nc.scalar.dma_start(onesf_sb[:], onesf_d.ap())
        onesr_sb = persist.tile([1, 128], F32R)
        nc.scalar.dma_start(onesr_sb[:], onesr_d.ap())
        id_sb = persist.tile([128, 128], F32)
        nc.sync.dma_start(id_sb[:], id_d.ap())
        swap_sb = persist.tile([128, 128], F32R)
        nc.sync.dma_start(swap_sb[:], swap_d.ap())
        mask_sb = persist.tile([128, CTA // 128, CTA], BF16)
        eps_sb = persist.tile([128, 1], F32)
        nc.vector.memset(eps_sb[:], EPS)

        # ---- phase 0: quantize own 256-token slice ----------------------
        amax4 = work.tile([128, 4, 2], F32, name="amax4", bufs=1)
        for q in range(4):
            tb, hf = q // 2, q % 2
            xtmq = work.tile([128, 8, CQ], F32, name="xtg", bufs=2, tag="zb")
            enq = nc.sync if q % 2 == 0 else nc.scalar
            enq.dma_start(
                xtmq[:].rearrange("p a t -> p (a t)"),
                xtm_d.ap()[tb, :, hf * (DIM // 2):(hf + 1) * (DIM // 2)])
            xa = work.tile([128, 8, CQ], F32, name="xa", bufs=2, tag="zb")
            nc.scalar.activation(xa[:].rearrange("p a t -> p (a t)"),
                                 xtmq[:].rearrange("p a t -> p (a t)"), AF.Abs)
            nc.vector.tensor_reduce(amax4[:, q, 0:1],
                                    xa[:].rearrange("p a t -> p (a t)"),
                                    axis=AX.X, op=OP.max)
        absmax = work.tile([128, 2], F32, name="absmax", bufs=1)
        for tb in range(2):
            nc.vector.tensor_reduce(
                absmax[:, tb:tb + 1],
                amax4[:, 2 * tb:2 * tb + 2, 0:1].rearrange("p a b -> p (a b)"),
                axis=AX.X, op=OP.max)
        nc.vector.tensor_scalar_max(absmax[:], absmax[:], 1e-5)
        scoef = work.tile([128, 2, 2], F32, name="scoef", bufs=1)
        nc.vector.reciprocal(scoef[:, 0, :], absmax[:])
        nc.vector.tensor_scalar_mul(scoef[:, 0, :], scoef[:, 0, :], 127.0)
        nc.vector.tensor_scalar_mul(scoef[:, 1, :], absmax[:], 1.0 / 127.0)
        statq_d = dram.tile([2, CQ], F32, name="statq_d")
        nc.sync.dma_start(
            statq_d[:].rearrange("r (th tl) -> tl r th", tl=128), scoef[:])
        s_bc = work.tile([128, CQ], F32, name="s_bc", bufs=1, tag="sb1")
        nc.scalar.dma_start(s_bc[:], statq_d[0:1, :].to_broadcast((128, CQ)))
        xq_own_f = dram.tile([SZB + 2048], I8, name="xq_own_f")
        nc.scalar.dma_start(
            xq_own_f[SZB:].bitcast(F32)
            .rearrange("(r th tl) -> tl r th", tl=128, th=2), scoef[:])
        xq_own_d = xq_own_f[0:SZB].rearrange("(p dc t) -> p dc t", p=128, dc=DCH)
        for g in range(4):
            xtg = work.tile([128, 8, CQ], F32, name="xtg", bufs=2, tag="zb")
            eng = nc.sync if g % 2 == 0 else nc.scalar
            eng.dma_start(
                xtg[:], xt_d.ap()[g * 1024:(g + 1) * 1024, :]
                .rearrange("(dc p) t -> p dc t", p=128))
            xqg = work.tile([128, 8, CQ], I8, name="xqg", bufs=2, tag="sq2")
            for dc in range(8):
                t1 = work.tile([128, CQ], F32, name="t1", bufs=1, tag="tq1")
                nc.vector.tensor_tensor(t1[:], xtg[:, dc, :], s_bc[:], OP.mult)
                t2 = work.tile([128, CQ], F32, name="t2", bufs=2, tag="y4")
                nc.scalar.activation(t2[:], t1[:], AF.Copy, bias=MAGIC)
                nc.scalar.activation(xqg[:, dc, :], t2[:], AF.Copy, bias=-MAGIC)
            nc.sync.dma_start(xq_own_d[:, g * 8:(g + 1) * 8, :], xqg[:])
        xq_g = dram.tile([NCORES, SZB + 2048], I8, name="xq_g",
                         addr_space="Shared")
        nc.gpsimd.collective_compute(
            "AllGather", OP.bypass, replica_groups=[list(range(NCORES))],
            ins=[xq_own_f[:].opt()], outs=[xq_g[:].opt()])
        for i in range(4):
            nc.sync.dma_start(
                wq_sb[:, i * 8:(i + 1) * 8, :],
                wq_d.ap()[i * 1024:(i + 1) * 1024, :]
                .rearrange("(dc p) j -> p dc j", p=128))
        nc.sync.dma_start(mask_sb[:], mask_d.ap())

        if phase < 2:
            nc.gpsimd.dma_start(yt_d.ap()[0:128, 0:CQ], s_bc[:])
            nc.compile()
            return nc

        # ---- main loop over 512-token chunks ----------------------------
        tail_state = {}
        for cc in range(NCH):
            t0 = cc * CTA
            tsl = slice(t0, t0 + CTA)
            zq_c = work.tile([128, HPG, CTA], BF16, name="zq_c", bufs=2)
            z_c = work.tile([128, HPG, CTA], F32, name="z_c", bufs=2, tag="zb")
            if cc == 0:
                pre = _loads(nc, work, xq_g, None, cosf_d, sinf_d, 0)
            xq_c, sinv_bc, cos_ch, sin_ch = pre

            # -- QKV projection (N=512) --
            q_pre = work.tile([128, HPG, CTA], F32R, name="q_pre", bufs=1)
            k_pre = work.tile([128, CTA], F32R, name="k_pre", bufs=1)
            v_pre = work.tile([128, CTA], F32, name="v_pre", bufs=1)
            for jc in range(6):
                pq = ps_a.tile([128, CTA], F32, name="pq", tag="ps1")
                for dc in range(DCH):
                    nc.tensor.matmul(pq[:], wq_sb[:, dc, jc * 128:(jc + 1) * 128],
                                     xq_c[:, dc, :], start=(dc == 0),
                                     stop=(dc == DCH - 1))
                dst = (q_pre[:, jc, :] if jc < HPG
                       else (k_pre[:] if jc == HPG else v_pre[:]))
                nc.vector.tensor_tensor(dst, pq[:], sinv_bc[:], OP.mult)

            # -- V transpose to token-major (raw v, bf16) --
            for b in range(KB):
                vp = misc_ps.tile([128, 128], F32, name="vp", tag="mc")
                nc.tensor.transpose(vp[:], v_pre[:, b * 128:(b + 1) * 128], id_sb[:])
                nc.vector.tensor_copy(V_sb[:, t0 // 128 + b, :], vp[:])

            # -- RoPE (half-swap via PE permutation matmul) --
            q_sb = work.tile([128, HPG, CTA], F32R, name="q_sb", bufs=1)
            for r in range(5):
                srcr = k_pre[:] if r == HPG else q_pre[:, r, :]
                dst = K_sb[:, tsl] if r == HPG else q_sb[:, r, :]
                xs = misc_ps.tile([128, CTA], F32, name="xs", tag="mc")
                nc.tensor.matmul(xs[:], swap_sb[:], srcr, start=True, stop=True)
                tsc = work.tile([128, CTA], F32, name="tsc", bufs=1, tag="h1")
                nc.vector.tensor_tensor(tsc[:], xs[:], sin_ch[:], OP.mult)
                xc = work.tile([128, CTA], F32, name="xc", bufs=1, tag="h2")
                nc.vector.tensor_tensor(xc[:], srcr.bitcast(F32), cos_ch[:],
                                        OP.mult)
                nc.vector.tensor_tensor(dst, xc[:], tsc[:], OP.add)

            if phase < 3:
                nc.gpsimd.dma_start(yt_d.ap()[0:128, tsl],
                                    q_sb[:, 0, :].bitcast(F32))
                continue

            # -- prefetch next chunk + deferred tail of previous chunk --
            if cc + 1 < NCH:
                pre = _loads(nc, work, xq_g, None, cosf_d, sinf_d, cc + 1)
            if cc >= 1 and phase >= 4:
                _tail(nc, work, l_ps, misc_ps, ones8_sb, onesr_sb, eps_sb,
                      tail_state[cc - 1])

            # -- attention (head pairs; kb loop software-pipelined) --
            nkb = KB * (cc + 1)
            for hp in range(2):
                lp = l_ps.tile([2, CTA], F32, name="lp", tag="lp")
                pvs = []
                for h in (2 * hp, 2 * hp + 1):
                    pv = ps_b.tile([128, CTA], F32, name="pv", tag="pv")
                    pvs.append(pv)
                    sp_next = ps_a.tile([128, CTA], F32, name="sp", tag="ps1")
                    nc.tensor.matmul(sp_next[:], K_sb[:, 0:128],
                                     q_sb[:, h, :], start=True, stop=True)
                    for kb in range(nkb):
                        sp = sp_next
                        if kb + 1 < nkb:
                            sp_next = ps_a.tile([128, CTA], F32, name="sp",
                                                tag="ps1")
                            nc.tensor.matmul(
                                sp_next[:],
                                K_sb[:, (kb + 1) * 128:(kb + 2) * 128],
                                q_sb[:, h, :], start=True, stop=True)
                        P = pwork.tile([128, CTA], BF16, name="P", tag="pp")
                        diag = kb - (nkb - KB)
                        if diag < 0:
                            nc.scalar.activation(P[:], sp[:], AF.Exp, scale=SCALE)
                        else:
                            Pt = pwork.tile([128, CTA], BF16, name="Pt", tag="pp")
                            nc.scalar.activation(Pt[:], sp[:], AF.Exp, scale=SCALE)
                            nc.vector.tensor_tensor(P[:], Pt[:],
                                                    mask_sb[:, diag, :], OP.mult)
                        nc.tensor.matmul(pv[:], V_sb[:, kb, :], P[:],
                                         start=(kb == 0), stop=(kb == nkb - 1))
                        nc.tensor.matmul(lp[:], onesh_sb[:, h % 2, :], P[:],
                                         start=(h == 2 * hp and kb == 0),
                                         stop=(h == 2 * hp + 1 and kb == nkb - 1),
                                         skip_group_check=True)
                lrows = work.tile([2, CTA], F32, name="lrows", bufs=1)
                nc.scalar.activation(lrows[:], lp[:], AF.Copy)
                invr = work.tile([2, CTA], F32R, name="invr", bufs=1)
                with nc.allow_low_precision(reason="f32r bcast rhs"):
                    nc.vector.reciprocal(invr[:], lrows[:])
                for i, h in enumerate((2 * hp, 2 * hp + 1)):
                    ib = misc_ps.tile([128, CTA], F32, name="ib", tag="mc")
                    nc.tensor.matmul(ib[:], selrms_sb[:, h, :], invr[:],
                                     start=True, stop=True)
                    ibs = work.tile([128, CTA], F32, name="ibs", bufs=1)
                    nc.scalar.activation(ibs[:], ib[:], AF.Copy)
                    nc.vector.tensor_tensor(z_c[:, h, :], pvs[i][:],
                                            ibs[:], OP.mult)

            if phase < 4:
                nc.gpsimd.dma_start(yt_d.ap()[0:128, tsl], z_c[:, 0, :])
                continue

            if cc >= 1 and phase >= 4:
                _wo_pass(nc, work, ps_b, wo_d, tail_state[cc - 1][3], yt_d,
                         cc - 1)

            # -- stats: ssq + max(z^2) per token --
            ssq = l_ps.tile([1, CTA], F32, name="ssq", tag="lp")
            hsq = work.tile([128, CTA], F32, name="hsq", bufs=1, tag="h1")
            hsq2 = work.tile([128, CTA], F32, name="hsq2", bufs=1, tag="h2")
            for h in range(HPG):
                sqh = work.tile([128, CTA], F32R, name="sqh", bufs=2, tag="sq2")
                nc.scalar.activation(sqh[:], z_c[:, h, :], AF.Square)
                nc.tensor.matmul(ssq[:], onesf_sb[:], sqh[:],
                                 start=(h == 0), stop=(h == HPG - 1))
                if h == 0:
                    nc.vector.tensor_copy(hsq[:], sqh[:].bitcast(F32))
                elif h == 1:
                    nc.vector.tensor_copy(hsq2[:], sqh[:].bitcast(F32))
                else:
                    dsth = hsq if h == 2 else hsq2
                    nc.vector.tensor_tensor(dsth[:], dsth[:],
                                            sqh[:].bitcast(F32), OP.max)
            nc.vector.tensor_tensor(hsq[:], hsq[:], hsq2[:], OP.max)
            mx2c = work.tile([128, KB], F32, name="mx2c", bufs=1)
            for b in range(KB):
                tp = misc_ps.tile([128, 128], F32, name="tp", tag="mc")
                nc.tensor.transpose(tp[:], hsq[:, b * 128:(b + 1) * 128], id_sb[:])
                nc.vector.tensor_reduce(mx2c[:, b:b + 1], tp[:], axis=AX.X,
                                        op=OP.max)
            ssq_sb = work.tile([1, CTA], F32, name="ssq_sb", bufs=1, tag="sb1")
            nc.scalar.activation(ssq_sb[:], ssq[:], AF.Copy)
            mx2t = misc_ps.tile([KB, 128], F32, name="mx2t", tag="mc")
            nc.tensor.transpose(mx2t[:], mx2c[:], id_sb[:])
            mx2r = work.tile([KB, 128], F32, name="mx2r")
            nc.scalar.activation(mx2r[:], mx2t[:], AF.Copy)
            statr_d = dram.tile([2, CTA], F32, name=f"statr_d{cc}")
            nc.sync.dma_start(statr_d[0:1, :], ssq_sb[:])
            nc.scalar.dma_start(
                statr_d[1:2, :].rearrange("r (th tl) -> (r th) tl", tl=128),
                mx2r[:])
            statr_g = dram.tile([NCORES, 2, CTA], F32, name=f"statr_g{cc}",
                                addr_space="Shared")
            nc.gpsimd.collective_compute(
                "AllGather", OP.bypass, replica_groups=[list(range(NCORES))],
                ins=[statr_d[:].opt()], outs=[statr_g[:].opt()])
            g8 = work.tile([NCORES, CTA], F32R, name="g8", bufs=1)
            nc.gpsimd.dma_start(g8[:], statr_g[:, 0, :])
            m8 = work.tile([NCORES, CTA], F32, name="m8", bufs=1)
            nc.gpsimd.dma_start(m8[:], statr_g[:, 1, :])
            tail_state[cc] = (g8, m8, z_c, zq_c)
        if phase >= 4:
            _tail(nc, work, l_ps, misc_ps, ones8_sb, onesr_sb, eps_sb,
                  tail_state[NCH - 1])
            _wo_pass(nc, work, ps_b, wo_d, tail_state[NCH - 1][3], yt_d,
                     NCH - 1)

    nc.compile()
    return nc


def _loads(nc, work, xq_g, statq_g, cosf_d, sinf_d, cc):
    """Prefetch chunk cc's xq (int8->bf16 cast), sinv broadcast, rope rows."""
    t0 = cc * CTA
    xq_c = work.tile([128, DCH, CTA], BF16, name="xq_c", bufs=1)
    for g in range(2):
        nc.gpsimd.dma_start(
            xq_c[:, :, g * CQ:(g + 1) * CQ],
            xq_g[2 * cc + g, 0:SZB].rearrange("(p dc t) -> p dc t",
                                              p=128, dc=DCH))
    sinv_bc = work.tile([128, CTA], F32, name="sinv_bc", bufs=1)
    for g in range(2):
        nc.sync.dma_start(
            sinv_bc[:, g * CQ:(g + 1) * CQ],
            xq_g[2 * cc + g, SZB + 1024:].bitcast(F32)
            .rearrange("t -> () t").to_broadcast((128, CQ)))
    cos_ch = work.tile([128, CTA], F32, name="cos_ch", bufs=1)
    nc.scalar.dma_start(cos_ch[:], cosf_d.ap()[:, t0:t0 + CTA])
    sin_ch = work.tile([128, CTA], F32, name="sin_ch", bufs=1)
    nc.scalar.dma_start(sin_ch[:], sinf_d.ap()[:, t0:t0 + CTA])
    return xq_c, sinv_bc, cos_ch, sin_ch


def _tail(nc, work, l_ps, misc_ps, ones8_sb, onesr_sb, eps_sb, st):
    """Global stats -> qf/c2 rows (p0) -> quantize z into zq (bf16)."""
    g8, m8, z_c, zqt = st
    ssqt = l_ps.tile([1, CTA], F32, name="ssqt", tag="lp")
    nc.tensor.matmul(ssqt[:], ones8_sb[:], g8[:], start=True, stop=True)
    m8r = work.tile([NCORES, CTA], F32, name="m8r", bufs=1, tag="r1")
    nc.gpsimd.partition_all_reduce(m8r[:], m8[:], channels=NCORES,
                                   reduce_op=bass_isa.ReduceOp.max)
    sroot = work.tile([1, CTA], F32, name="sroot", bufs=1, tag="r0")
    nc.scalar.activation(sroot[:], ssqt[:], AF.Sqrt, scale=1.0 / DIM,
                         bias=eps_sb[0:1, :])
    gmaxr = work.tile([1, CTA], F32, name="gmaxr", bufs=1)
    nc.scalar.activation(gmaxr[:], m8r[0:1, :], AF.Sqrt)
    rrow = work.tile([1, CTA], F32, name="rrow", bufs=1)
    nc.vector.reciprocal(rrow[:], sroot[:])
    qfr0 = work.tile([1, CTA], F32, name="qfr0", bufs=1, tag="r0")
    nc.vector.reciprocal(qfr0[:], gmaxr[:])
    qfr = work.tile([1, CTA], F32R, name="qfr", bufs=1)
    with nc.allow_low_precision(reason="f32r bcast rhs"):
        nc.vector.tensor_scalar_mul(qfr[:], qfr0[:], 127.0)
    c2r0 = work.tile([1, CTA], F32, name="c2r0", bufs=1, tag="r1")
    nc.vector.tensor_tensor(c2r0[:], rrow[:], gmaxr[:], OP.mult)
    c2r = work.tile([1, CTA], F32R, name="c2r", bufs=1)
    with nc.allow_low_precision(reason="f32r bcast rhs"):
        nc.vector.tensor_scalar_mul(c2r[:], c2r0[:], 1.0 / 127.0)
    qf_bc = work.tile([128, CTA], F32, name="qf_bc", bufs=1)
    c2_bc = work.tile([128, CTA], F32, name="c2_bc", bufs=1)
    for rowt, dstb in ((qfr, qf_bc), (c2r, c2_bc)):
        bp = misc_ps.tile([128, CTA], F32, name="bp", tag="mc")
        nc.tensor.matmul(bp[:], onesr_sb[:], rowt[:], start=True, stop=True)
        nc.scalar.activation(dstb[:], bp[:], AF.Copy)
    for h in range(HPG):
        tq = work.tile([128, CTA], F32, name="tq", bufs=1, tag="tq1")
        nc.vector.tensor_tensor(tq[:], z_c[:, h, :], qf_bc[:], OP.mult)
        nc.vector.tensor_scalar(tq[:], tq[:], MAGIC, None, OP.add)
        nc.vector.scalar_tensor_tensor(zqt[:, h, :], tq[:],
                                       -MAGIC, c2_bc[:], OP.add, OP.mult)


def _wo_pass(nc, work, wo_ps, wo_d, zq, yt_d, cc):
    """y[:, tsl] partial = wo.T @ zq(chunk cc); bf16 out, N=512."""
    t0 = cc * CTA
    for g4 in range(DCH // 4):
        wo_t = work.tile([128, HPG, 512], BF16, name="wo_t", bufs=2)
        nc.scalar.dma_start(
            wo_t[:], wo_d.ap()[:, g4 * 512:(g4 + 1) * 512]
            .rearrange("(jc p) i -> p jc i", p=128))
        y4 = work.tile([128, 4, CTA], BF16, name="y4", bufs=2, tag="y4")
        for icc in range(4):
            yp = wo_ps.tile([128, CTA], F32, name="yp", tag="pv")
            for jc in range(HPG):
                nc.tensor.matmul(
                    yp[:], wo_t[:, jc, icc * 128:(icc + 1) * 128],
                    zq[:, jc, :],
                    start=(jc == 0), stop=(jc == HPG - 1))
            nc.scalar.activation(y4[:, icc, :], yp[:], AF.Copy)
        eng = nc.sync if g4 % 2 == 0 else nc.scalar
        eng.dma_start(
            yt_d.ap()[g4 * 512:(g4 + 1) * 512, t0:t0 + CTA]
            .rearrange("(icc p) t -> p icc t", p=128), y4[:])


# ======================= host-side preparation ==========================

def _rope_tables(T):
    inv = THETA ** (-np.arange(0, HD, 2, dtype=np.float64) / HD)
    pos = np.arange(T, dtype=np.float64)
    ang = (pos[None, :] * inv[:, None]).astype(np.float32)  # [64, T]
    cos = np.cos(ang.astype(np.float64)).astype(np.float32)
    sin = np.sin(ang.astype(np.float64)).astype(np.float32)
    cosf = np.concatenate([cos, cos], axis=0)
    sinf = np.concatenate([-sin, sin], axis=0)
    return np.ascontiguousarray(cosf), np.ascontiguousarray(sinf)


def _perm_rope():
    return np.concatenate([np.arange(0, HD, 2), np.arange(1, HD, 2)])


def make_inputs(x, w_qkv, w_o, rms_w, T=2048):
    import ml_dtypes
    bf16 = ml_dtypes.bfloat16
    perm = _perm_rope()
    cosf, sinf = _rope_tables(T)
    KB = CTA // 128
    mask = np.zeros((128, KB, CTA), dtype=np.float32)
    kt = np.arange(128)[:, None]
    qt = np.arange(CTA)[None, :]
    for d in range(KB):
        mask[:, d, :] = (kt + 128 * d <= qt)
    ident = np.eye(128, dtype=np.float32)
    swap64 = np.roll(np.eye(128, dtype=np.float32), 64, axis=0)
    onesh = np.zeros((128, 2, 2), dtype=np.float32)
    for i in range(2):
        onesh[:, i, i] = 1.0
    ones8 = np.ones((NCORES, 1), dtype=np.float32)
    onesf = np.ones((128, 1), dtype=np.float32)
    onesr = np.ones((1, 128), dtype=np.float32)

    wq_full = w_qkv[:NH * HD].reshape(NKV, HPG, HD, DIM)
    wk_full = w_qkv[NH * HD:NH * HD + NKV * HD].reshape(NKV, HD, DIM)
    wv_full = w_qkv[NH * HD + NKV * HD:].reshape(NKV, HD, DIM)

    in_maps = []
    for c in range(NCORES):
        wq_c = wq_full[c][:, perm, :].reshape(JQ, DIM)
        wk_c = wk_full[c][perm, :]
        wv_c = wv_full[c]
        w_cat = np.concatenate([wq_c, wk_c, wv_c], axis=0)   # [768, DIM]
        wo_c = w_o[:, c * JQ:(c + 1) * JQ]                   # [DIM, 512]
        rms_c = rms_w[c * JQ:(c + 1) * JQ]
        selrms = np.zeros((2, HPG, 128), dtype=np.float32)
        for h in range(HPG):
            selrms[h % 2, h, :] = rms_c[h * 128:(h + 1) * 128]
        xsl = x[c * CQ:(c + 1) * CQ]                         # [256, DIM]
        in_maps.append(dict(
            xtm=np.ascontiguousarray(xsl.reshape(2, 128, DIM)),
            xt=np.ascontiguousarray(xsl.T),                  # [DIM, 256]
            wq=np.ascontiguousarray(w_cat.T).astype(bf16),   # [DIM, 768]
            wo=np.ascontiguousarray(wo_c.T).astype(bf16),    # [512, DIM]
            selrms=selrms,
            onesh=onesh.astype(bf16), ones8=ones8, onesf=onesf, onesr=onesr,
            cosf=cosf, sinf=sinf,
            maskt=mask.astype(bf16), ident=ident, swap64=swap64,
        ))
    return in_maps


def combine_outputs(results):
    acc = np.zeros((DIM, 2048), dtype=np.float64)
    for r in results:
        acc += np.asarray(r["yt"], dtype=np.float64)
    return np.ascontiguousarray(acc.T.astype(np.float32))


def _install_axon_profile_shim():
    import types
    try:
        import antenv.axon_hooks  # noqa: F401
        return
    except ImportError:
        pass
    try:
        import antenv
        from trn_agent_boot.trn_boot import _ntff_profile_via_ctypes
    except ImportError:
        return
    so_path = "/opt/axon/libaxon_pjrt.so"
    import os
    if not os.path.exists(so_path):
        return
    mod = types.ModuleType("antenv.axon_hooks")
    _hook = {"fn": _ntff_profile_via_ctypes(so_path)}
    mod.set_axon_ntff_profile_hook = lambda fn: _hook.__setitem__("fn", fn)
    mod.get_axon_ntff_profile_hook = lambda: _hook["fn"]
    sys.modules["antenv.axon_hooks"] = mod
    antenv.axon_hooks = mod


_install_axon_profile_shim()


# ======================= public entry point =============================

_NC_CACHE = {}


def _get_nc(T, phase=99):
    key = (T, phase)
    if key not in _NC_CACHE:
        _NC_CACHE[key] = build_kernel(T=T, phase=phase)
    return _NC_CACHE[key]


def kernel(x, w_qkv, w_o, rms_w, cache_k=None, cache_v=None, **_ignored):
    from concourse.bass_utils import run_bass_kernel_spmd
    x = np.asarray(x, dtype=np.float32)
    T = x.shape[0]
    nc = _get_nc(T)
    in_maps = make_inputs(x, np.asarray(w_qkv, np.float32),
                          np.asarray(w_o, np.float32),
                          np.asarray(rms_w, np.float32), T=T)
    res = run_bass_kernel_spmd(nc, in_maps, core_ids=list(range(NCORES)))
    return combine_outputs(res.results)


def kernel_profiled(x, w_qkv, w_o, rms_w, cache_k=None, cache_v=None, phase=99):
    from concourse.bass_utils import run_bass_kernel_spmd
    x = np.asarray(x, dtype=np.float32)
    T = x.shape[0]
    nc = _get_nc(T, phase)
    in_maps = make_inputs(x, np.asarray(w_qkv, np.float32),
                          np.asarray(w_o, np.float32),
                          np.asarray(rms_w, np.float32), T=T)
    res = run_bass_kernel_spmd(nc, in_maps, core_ids=list(range(NCORES)),
                               trace=True)
    return combine_outputs(res.results), res.exec_time_ns
